# revision 1
# baseline (speedup 1.0000x reference)
"""DeepSeekMoE block on 8 Trainium2 NeuronCores.

Sharding: expert-parallel — core e owns expert e's FFN (up_w[e]/down_w[e]);
tokens are dispatched to expert cores by host-side top-2 gating (the gate
matmul is 0.03% of total FLOPs).  The shared expert is token-parallel:
core e also runs the shared FFN for tokens [e*256, (e+1)*256).

Device kernel per core (SPMD):
  hact = gelu(up_w[e].T-tiles @ xT + up_b[e])        # [I, cap] layout
  eoT  = 0.1 * (down_w[e]-tiles @ hact + down_b[e])  # [H, cap]
  same for the shared expert on its 256-token slice.
Matmuls run in bf16 (fp32 PSUM accumulate); set DTYPE="f32r" for
full-rate fp32 (2x DMA traffic, ~10x lower error).

Host: gating/top-k (fp64 scores, fp32 combine weights), scatter-add of the
two expert contributions per token + shared path, row max-abs normalize.
"""
import sys
sys.path.insert(0, '/opt/trn_rl_repo')
import numpy as np
from contextlib import ExitStack

H = 1024
I = 4096
E = 8
TOPK = 2
B, S = 2, 1024
T = B * S            # 2048 tokens
CAP = 544            # routed-token capacity per expert core (max count is 542)
TS = T // E          # shared-expert tokens per core = 256
HC = H // 128        # 8 h-chunks
IC = I // 128        # 32 i-chunks
DTYPE = "bf16"       # "bf16" | "f32r"
PHASES = ("routed", "shared")

if DTYPE == "bf16":
    BLK_R = (512, 32)
    BLK_S = (256,)
else:
    BLK_R = (288, 256)   # f32r needs moving dim >=256 for full rate
    BLK_S = (256,)

_COMPILED = {}


def _build_nc():
    from concourse import bacc, tile, mybir

    F32 = mybir.dt.float32
    CDT = mybir.dt.bfloat16 if DTYPE == "bf16" else mybir.dt.float32r
    GELU = mybir.ActivationFunctionType.Gelu
    IDENT = mybir.ActivationFunctionType.Identity

    nc = bacc.Bacc("TRN2", target_bir_lowering=False, debug=False, num_devices=E)

    xT_d = nc.dram_tensor("xT", [128, HC * CAP], CDT, kind="ExternalInput")
    xsT_d = nc.dram_tensor("xsT", [128, HC * TS], CDT, kind="ExternalInput")
    upw_d = nc.dram_tensor("upw", [128, IC * HC * 128], CDT, kind="ExternalInput")
    dnw_d = nc.dram_tensor("dnw", [128, HC * IC * 128], CDT, kind="ExternalInput")
    supw_d = nc.dram_tensor("supw", [128, IC * HC * 128], CDT, kind="ExternalInput")
    sdnw_d = nc.dram_tensor("sdnw", [128, HC * IC * 128], CDT, kind="ExternalInput")
    upb_d = nc.dram_tensor("upb", [128, IC], F32, kind="ExternalInput")
    supb_d = nc.dram_tensor("supb", [128, IC], F32, kind="ExternalInput")
    dnb_d = nc.dram_tensor("dnb", [128, HC], F32, kind="ExternalInput")
    sdnb_d = nc.dram_tensor("sdnb", [128, HC], F32, kind="ExternalInput")
    eoT_d = nc.dram_tensor("eoT", [HC, 128, CAP], F32, kind="ExternalOutput")
    soT_d = nc.dram_tensor("soT", [HC, 128, TS], F32, kind="ExternalOutput")

    with tile.TileContext(nc) as tc, ExitStack() as ctx:
        pool = ctx.enter_context(tc.tile_pool(name="sbuf", bufs=1))
        uwpool = ctx.enter_context(tc.tile_pool(name="uwstream", bufs=4))
        dwpool = ctx.enter_context(tc.tile_pool(name="dwstream", bufs=3))
        hpool_r = ctx.enter_context(tc.tile_pool(name="hact_r", bufs=IC))
        hpool_s = ctx.enter_context(tc.tile_pool(name="hact_s", bufs=IC))
        opool = ctx.enter_context(tc.tile_pool(name="outs", bufs=6))
        upps = ctx.enter_context(tc.tile_pool(name="upps", bufs=2, space="PSUM"))
        dnps = ctx.enter_context(tc.tile_pool(name="dnps", bufs=2, space="PSUM"))

        # resident activations + biases
        xT_t = pool.tile([128, HC * CAP], CDT, tag="xT")
        for hc in range(HC):   # chunked so the first matmuls start sooner
            nc.sync.dma_start(xT_t[:, hc * CAP:(hc + 1) * CAP],
                              xT_d.ap()[:, hc * CAP:(hc + 1) * CAP])
        xsT_t = pool.tile([128, HC * TS], CDT, tag="xsT")
        nc.sync.dma_start(xsT_t[:], xsT_d.ap()[:])
        upb_t = pool.tile([128, IC], F32, tag="upb")
        nc.sync.dma_start(upb_t[:], upb_d.ap()[:])
        supb_t = pool.tile([128, IC], F32, tag="supb")
        nc.sync.dma_start(supb_t[:], supb_d.ap()[:])
        dnb_t = pool.tile([128, HC], F32, tag="dnb")
        nc.sync.dma_start(dnb_t[:], dnb_d.ap()[:])
        sdnb_t = pool.tile([128, HC], F32, tag="sdnb")
        nc.sync.dma_start(sdnb_t[:], sdnb_d.ap()[:])

        def ffn(x_t, w_up_d, w_dn_d, b_up_t, b_dn_t, out_d, ntok, blocks, hpool):
            """One expert FFN over `ntok` token columns of x_t ([128, HC*ntok])."""
            # --- up projection + gelu: hact[ic] = gelu(up_w.T @ x + b) ---
            hacts = []
            for ic in range(IC):
                uw = uwpool.tile([128, HC * 128], CDT, tag="upw")
                nc.sync.dma_start(
                    uw[:], w_up_d.ap()[:, ic * HC * 128:(ic + 1) * HC * 128])
                ht = hpool.tile([128, ntok], CDT, tag="hact")
                t0 = 0
                for nb in blocks:
                    ps = upps.tile([128, nb], F32, tag="upps")
                    for hc in range(HC):
                        nc.tensor.matmul(
                            ps[:],
                            uw[:, hc * 128:(hc + 1) * 128],
                            x_t[:, hc * ntok + t0: hc * ntok + t0 + nb],
                            start=(hc == 0), stop=(hc == HC - 1),
                        )
                    if DTYPE == "bf16":
                        nc.scalar.activation(
                            ht[:, t0:t0 + nb], ps[:], GELU, bias=b_up_t[:, ic:ic + 1])
                    else:
                        # ScalarE cannot round to f32r (HW garbage) -> gelu to f32
                        # staging tile, DVE copy performs the f32r rounding.
                        g32 = opool.tile([128, nb], F32, tag="g32")
                        nc.scalar.activation(
                            g32[:], ps[:], GELU, bias=b_up_t[:, ic:ic + 1])
                        nc.vector.tensor_copy(ht[:, t0:t0 + nb], g32[:])
                    t0 += nb
                hacts.append(ht)

            # --- down projection: out[hb] = 0.1 * (dn_w.T @ hact + b) ---
            for hb in range(HC):
                dw = dwpool.tile([128, IC * 128], CDT, tag="dnw")
                nc.sync.dma_start(
                    dw[:], w_dn_d.ap()[:, hb * IC * 128:(hb + 1) * IC * 128])
                t0 = 0
                for nb in blocks:
                    ps = dnps.tile([128, nb], F32, tag="dnps")
                    for ic in range(IC):
                        nc.tensor.matmul(
                            ps[:],
                            dw[:, ic * 128:(ic + 1) * 128],
                            hacts[ic][:, t0:t0 + nb],
                            start=(ic == 0), stop=(ic == IC - 1),
                        )
                    ot = opool.tile([128, nb], F32, tag="out")
                    nc.scalar.activation(
                        ot[:], ps[:], IDENT, bias=b_dn_t[:, hb:hb + 1], scale=0.1)
                    nc.sync.dma_start(out_d.ap()[hb, :, t0:t0 + nb], ot[:])
                    t0 += nb

        if "routed" in PHASES:
            ffn(xT_t, upw_d, dnw_d, upb_t, dnb_t, eoT_d, CAP, BLK_R, hpool_r)
        if "shared" in PHASES:
            ffn(xsT_t, supw_d, sdnw_d, supb_t, sdnb_t, soT_d, TS, BLK_S, hpool_s)

    nc.compile()
    return nc


def _get_compiled():
    if "nc" not in _COMPILED:
        _COMPILED["nc"] = _build_nc()
    return _COMPILED["nc"]


def _np_cdt():
    if DTYPE == "bf16":
        import ml_dtypes
        return np.dtype(ml_dtypes.bfloat16)
    return np.dtype(np.float32)


def _pack_weight(w):
    """[K, N] -> [128, (N/128 chunks) x (K/128 subtiles) x 128] stream layout."""
    kdim, ndim = w.shape
    kc, nchunk = kdim // 128, ndim // 128
    return np.ascontiguousarray(
        w.reshape(kc, 128, nchunk, 128).transpose(1, 2, 0, 3)
    ).reshape(128, nchunk * kc * 128).astype(_np_cdt())


def _pack_tokens(xsel, cap):
    """[n, H] tokens -> [128, HC*cap] transposed h-chunked layout, zero pad."""
    n = xsel.shape[0]
    arr = np.zeros((128, HC, cap), np.float32)
    if n:
        arr[:, :, :n] = xsel.T.reshape(HC, 128, n).transpose(1, 0, 2)
    return np.ascontiguousarray(arr).reshape(128, HC * cap).astype(_np_cdt())


def _pack_bias(b, scale=1.0):
    """[N] -> [128, N/128] per-partition layout."""
    return np.ascontiguousarray(
        (np.asarray(b, np.float32) * scale).reshape(-1, 128).T.astype(np.float32))


def kernel(x, gate_w, bias, up_w, up_b, down_w, down_b,
           sw_up, sb_up, sw_down, sb_down):
    from concourse.bass_utils import run_bass_kernel_spmd

    x = np.asarray(x, np.float32)
    xf = x.reshape(T, H)

    # ---- host gating (fp64 scores for a stable top-k, fp32 combine weights)
    z64 = xf.astype(np.float64) @ np.asarray(gate_w, np.float64) \
        + np.asarray(bias, np.float64)
    scores64 = 1.0 / (1.0 + np.exp(-z64))
    top_idx = np.argsort(-scores64, axis=-1, kind="stable")[:, :TOPK]
    tsc = scores64[np.arange(T)[:, None], top_idx].astype(np.float32)
    wts = tsc / (tsc.sum(-1, keepdims=True) + np.float32(1e-6))   # [T, 2]

    # ---- token dispatch
    tok_lists = [np.where((top_idx == e).any(-1))[0] for e in range(E)]
    for e, tl in enumerate(tok_lists):
        if len(tl) > CAP:
            raise RuntimeError(f"expert {e} overflow: {len(tl)} > CAP={CAP}")

    supw = _pack_weight(np.asarray(sw_up, np.float32))
    sdnw = _pack_weight(np.asarray(sw_down, np.float32))
    supb = _pack_bias(sb_up)
    sdnb = _pack_bias(sb_down, scale=0.1)

    in_maps = []
    for e in range(E):
        in_maps.append({
            "xT": _pack_tokens(xf[tok_lists[e]], CAP),
            "xsT": _pack_tokens(xf[e * TS:(e + 1) * TS], TS),
            "upw": _pack_weight(np.asarray(up_w[e], np.float32)),
            "dnw": _pack_weight(np.asarray(down_w[e], np.float32)),
            "supw": supw,
            "sdnw": sdnw,
            "upb": _pack_bias(up_b[e]),
            "supb": supb,
            "dnb": _pack_bias(down_b[e], scale=0.1),
            "sdnb": sdnb,
        })

    nc = _get_compiled()
    res = run_bass_kernel_spmd(nc, in_maps, list(range(E)))

    # ---- host combine: scatter-add expert outputs, add shared, normalize
    out = np.zeros((T, H), np.float32)
    for e in range(E):
        soT = np.asarray(res.results[e]["soT"], np.float32)   # [HC, 128, TS]
        out[e * TS:(e + 1) * TS] = soT.reshape(H, TS).T
    for e in range(E):
        tl = tok_lists[e]
        if len(tl) == 0:
            continue
        eoT = np.asarray(res.results[e]["eoT"], np.float32)   # [HC, 128, CAP]
        eo = eoT.reshape(H, CAP)[:, :len(tl)].T               # [n, H]
        we = np.where(top_idx[tl, 0] == e, wts[tl, 0], wts[tl, 1]).astype(np.float32)
        out[tl] += we[:, None] * eo

    out /= (np.abs(out).max(-1, keepdims=True) + np.float32(1e-6))
    return out.reshape(B, S, H)



# revision 2
# speedup vs baseline: 1.2570x; 1.2570x over previous
"""DeepSeekMoE block on 8 Trainium2 NeuronCores.

Sharding: expert-parallel — core e owns expert e's FFN (up_w[e]/down_w[e]);
tokens are dispatched to expert cores by host-side top-2 gating (the gate
matmul is 0.03% of total FLOPs).  The shared expert is token-parallel:
core e also runs the shared FFN for tokens [e*256, (e+1)*256).

Routed capacity is CAP=512 (one PSUM bank of columns); the few token-pairs
beyond an expert's capacity are computed on the host in fp32 (~1% FLOPs).

Device kernel per core (SPMD), routed + shared INTERLEAVED per iteration so
the weight-stream DMA demand is flat (~200 GB/s) instead of alternating
145/305 GB/s phases:
  up:   for ic: hact_r[ic] = gelu(up_w[e][:,ic].T @ xT + up_b)   [128,512]
               hact_s[ic] = gelu(sw_up[:,ic].T  @ xsT + sb_up)   [128,256]
  down: for hb: eoT[hb] = 0.1*(dn_w tiles @ hact_r + dn_b)       [128,512]
               soT[hb] = 0.1*(sw_dn tiles @ hact_s + sb_dn)      [128,256]
Matmuls in bf16 (fp32 PSUM accumulate); outputs written as bf16.

Host: gating/top-k (fp64 scores, fp32 combine weights), overflow-pair FFN,
scatter-add of the expert contributions + shared path, row max-abs norm.
"""
import sys
sys.path.insert(0, '/opt/trn_rl_repo')
import numpy as np
from contextlib import ExitStack

H = 1024
I = 4096
E = 8
TOPK = 2
B, S = 2, 1024
T = B * S            # 2048 tokens
CAP = 512            # routed-token capacity per expert core (overflow -> host)
TS = T // E          # shared-expert tokens per core = 256
HC = H // 128        # 8 h-chunks
IC = I // 128        # 32 i-chunks
XCH = 4              # xT initial-DMA chunks

_COMPILED = {}


def _build_nc():
    from concourse import bacc, tile, mybir

    F32 = mybir.dt.float32
    CDT = mybir.dt.bfloat16
    GELU = mybir.ActivationFunctionType.Gelu
    IDENT = mybir.ActivationFunctionType.Identity

    nc = bacc.Bacc("TRN2", target_bir_lowering=False, debug=False, num_devices=E)

    xT_d = nc.dram_tensor("xT", [128, HC * CAP], CDT, kind="ExternalInput")
    xsT_d = nc.dram_tensor("xsT", [128, HC * TS], CDT, kind="ExternalInput")
    upw_d = nc.dram_tensor("upw", [128, IC * HC * 128], CDT, kind="ExternalInput")
    dnw_d = nc.dram_tensor("dnw", [128, HC * IC * 128], CDT, kind="ExternalInput")
    supw_d = nc.dram_tensor("supw", [128, IC * HC * 128], CDT, kind="ExternalInput")
    sdnw_d = nc.dram_tensor("sdnw", [128, HC * IC * 128], CDT, kind="ExternalInput")
    upb_d = nc.dram_tensor("upb", [128, IC], F32, kind="ExternalInput")
    supb_d = nc.dram_tensor("supb", [128, IC], F32, kind="ExternalInput")
    dnb_d = nc.dram_tensor("dnb", [128, HC], F32, kind="ExternalInput")
    sdnb_d = nc.dram_tensor("sdnb", [128, HC], F32, kind="ExternalInput")
    eoT_d = nc.dram_tensor("eoT", [HC, 128, CAP], CDT, kind="ExternalOutput")
    soT_d = nc.dram_tensor("soT", [HC, 128, TS], CDT, kind="ExternalOutput")

    with tile.TileContext(nc) as tc, ExitStack() as ctx:
        pool = ctx.enter_context(tc.tile_pool(name="sbuf", bufs=1))
        uwpool_r = ctx.enter_context(tc.tile_pool(name="uwr", bufs=4))
        uwpool_s = ctx.enter_context(tc.tile_pool(name="uws", bufs=4))
        dwpool_r = ctx.enter_context(tc.tile_pool(name="dwr", bufs=3))
        dwpool_s = ctx.enter_context(tc.tile_pool(name="dws", bufs=3))
        hpool_r = ctx.enter_context(tc.tile_pool(name="hact_r", bufs=IC))
        hpool_s = ctx.enter_context(tc.tile_pool(name="hact_s", bufs=IC))
        opool = ctx.enter_context(tc.tile_pool(name="outs", bufs=4))
        ups_r = ctx.enter_context(tc.tile_pool(name="upsr", bufs=2, space="PSUM"))
        ups_s = ctx.enter_context(tc.tile_pool(name="upss", bufs=2, space="PSUM"))
        dns_r = ctx.enter_context(tc.tile_pool(name="dnsr", bufs=2, space="PSUM"))
        dns_s = ctx.enter_context(tc.tile_pool(name="dnss", bufs=2, space="PSUM"))

        # resident activations + biases.  xT chunked so the first matmul
        # chain can start before the whole tile lands.
        xT_t = pool.tile([128, HC * CAP], CDT, tag="xT")
        xw = HC * CAP // XCH
        for xc in range(XCH):
            nc.sync.dma_start(xT_t[:, xc * xw:(xc + 1) * xw],
                              xT_d.ap()[:, xc * xw:(xc + 1) * xw])
        uw0_r = uwpool_r.tile([128, HC * 128], CDT, tag="upw")
        nc.sync.dma_start(uw0_r[:], upw_d.ap()[:, 0:HC * 128])
        xsT_t = pool.tile([128, HC * TS], CDT, tag="xsT")
        nc.sync.dma_start(xsT_t[:], xsT_d.ap()[:])
        uw0_s = uwpool_s.tile([128, HC * 128], CDT, tag="supw")
        nc.sync.dma_start(uw0_s[:], supw_d.ap()[:, 0:HC * 128])
        upb_t = pool.tile([128, IC], F32, tag="upb")
        nc.sync.dma_start(upb_t[:], upb_d.ap()[:])
        supb_t = pool.tile([128, IC], F32, tag="supb")
        nc.sync.dma_start(supb_t[:], supb_d.ap()[:])
        dnb_t = pool.tile([128, HC], F32, tag="dnb")
        nc.sync.dma_start(dnb_t[:], dnb_d.ap()[:])
        sdnb_t = pool.tile([128, HC], F32, tag="sdnb")
        nc.sync.dma_start(sdnb_t[:], sdnb_d.ap()[:])

        # ---- up projection + gelu, routed & shared interleaved per ic ----
        hacts_r, hacts_s = [], []
        for ic in range(IC):
            if ic == 0:
                uw_r, uw_s = uw0_r, uw0_s
            else:
                uw_r = uwpool_r.tile([128, HC * 128], CDT, tag="upw")
                nc.sync.dma_start(
                    uw_r[:], upw_d.ap()[:, ic * HC * 128:(ic + 1) * HC * 128])
                uw_s = uwpool_s.tile([128, HC * 128], CDT, tag="supw")
                nc.sync.dma_start(
                    uw_s[:], supw_d.ap()[:, ic * HC * 128:(ic + 1) * HC * 128])

            ps = ups_r.tile([128, CAP], F32, tag="upsr")
            for hc in range(HC):
                nc.tensor.matmul(
                    ps[:],
                    uw_r[:, hc * 128:(hc + 1) * 128],
                    xT_t[:, hc * CAP:(hc + 1) * CAP],
                    start=(hc == 0), stop=(hc == HC - 1),
                )
            ht = hpool_r.tile([128, CAP], CDT, tag="hact")
            nc.scalar.activation(ht[:], ps[:], GELU, bias=upb_t[:, ic:ic + 1])
            hacts_r.append(ht)

            ps = ups_s.tile([128, TS], F32, tag="upss")
            for hc in range(HC):
                nc.tensor.matmul(
                    ps[:],
                    uw_s[:, hc * 128:(hc + 1) * 128],
                    xsT_t[:, hc * TS:(hc + 1) * TS],
                    start=(hc == 0), stop=(hc == HC - 1),
                )
            ht = hpool_s.tile([128, TS], CDT, tag="shact")
            nc.scalar.activation(ht[:], ps[:], GELU, bias=supb_t[:, ic:ic + 1])
            hacts_s.append(ht)

        # ---- down projection, routed & shared interleaved per hb ----
        for hb in range(HC):
            dw_r = dwpool_r.tile([128, IC * 128], CDT, tag="dnw")
            nc.sync.dma_start(
                dw_r[:], dnw_d.ap()[:, hb * IC * 128:(hb + 1) * IC * 128])
            dw_s = dwpool_s.tile([128, IC * 128], CDT, tag="sdnw")
            nc.sync.dma_start(
                dw_s[:], sdnw_d.ap()[:, hb * IC * 128:(hb + 1) * IC * 128])

            ps = dns_r.tile([128, CAP], F32, tag="dnsr")
            for ic in range(IC):
                nc.tensor.matmul(
                    ps[:],
                    dw_r[:, ic * 128:(ic + 1) * 128],
                    hacts_r[ic][:],
                    start=(ic == 0), stop=(ic == IC - 1),
                )
            ot = opool.tile([128, CAP], CDT, tag="out")
            nc.scalar.activation(ot[:], ps[:], IDENT, bias=dnb_t[:, hb:hb + 1],
                                 scale=0.1)
            nc.sync.dma_start(eoT_d.ap()[hb, :, :], ot[:])

            ps = dns_s.tile([128, TS], F32, tag="dnss")
            for ic in range(IC):
                nc.tensor.matmul(
                    ps[:],
                    dw_s[:, ic * 128:(ic + 1) * 128],
                    hacts_s[ic][:],
                    start=(ic == 0), stop=(ic == IC - 1),
                )
            ot = opool.tile([128, TS], CDT, tag="sout")
            nc.scalar.activation(ot[:], ps[:], IDENT, bias=sdnb_t[:, hb:hb + 1],
                                 scale=0.1)
            nc.sync.dma_start(soT_d.ap()[hb, :, :], ot[:])

    nc.compile()
    return nc


def _get_compiled():
    if "nc" not in _COMPILED:
        _COMPILED["nc"] = _build_nc()
    return _COMPILED["nc"]


def _np_cdt():
    import ml_dtypes
    return np.dtype(ml_dtypes.bfloat16)


def _pack_weight(w):
    """[K, N] -> [128, (N/128 chunks) x (K/128 subtiles) x 128] stream layout."""
    kdim, ndim = w.shape
    kc, nchunk = kdim // 128, ndim // 128
    return np.ascontiguousarray(
        w.reshape(kc, 128, nchunk, 128).transpose(1, 2, 0, 3)
    ).reshape(128, nchunk * kc * 128).astype(_np_cdt())


def _pack_tokens(xsel, cap):
    """[n, H] tokens -> [128, HC*cap] transposed h-chunked layout, zero pad."""
    n = xsel.shape[0]
    arr = np.zeros((128, HC, cap), np.float32)
    if n:
        arr[:, :, :n] = xsel.T.reshape(HC, 128, n).transpose(1, 0, 2)
    return np.ascontiguousarray(arr).reshape(128, HC * cap).astype(_np_cdt())


def _pack_bias(b, scale=1.0):
    """[N] -> [128, N/128] per-partition layout."""
    return np.ascontiguousarray(
        (np.asarray(b, np.float32) * scale).reshape(-1, 128).T.astype(np.float32))


def _gelu(u):
    from scipy.special import erf
    return 0.5 * u * (1.0 + erf(u / np.sqrt(2.0)))


def kernel(x, gate_w, bias, up_w, up_b, down_w, down_b,
           sw_up, sb_up, sw_down, sb_down):
    from concourse.bass_utils import run_bass_kernel_spmd

    x = np.asarray(x, np.float32)
    xf = x.reshape(T, H)

    # ---- host gating (fp64 scores for a stable top-k, fp32 combine weights)
    z64 = xf.astype(np.float64) @ np.asarray(gate_w, np.float64) \
        + np.asarray(bias, np.float64)
    scores64 = 1.0 / (1.0 + np.exp(-z64))
    top_idx = np.argsort(-scores64, axis=-1, kind="stable")[:, :TOPK]
    tsc = scores64[np.arange(T)[:, None], top_idx].astype(np.float32)
    wts = tsc / (tsc.sum(-1, keepdims=True) + np.float32(1e-6))   # [T, 2]

    # ---- token dispatch: first CAP tokens per expert on device, rest host
    tok_lists, over_lists = [], []
    for e in range(E):
        tl = np.where((top_idx == e).any(-1))[0]
        tok_lists.append(tl[:CAP])
        over_lists.append(tl[CAP:])

    supw = _pack_weight(np.asarray(sw_up, np.float32))
    sdnw = _pack_weight(np.asarray(sw_down, np.float32))
    supb = _pack_bias(sb_up)
    sdnb = _pack_bias(sb_down, scale=0.1)

    in_maps = []
    for e in range(E):
        in_maps.append({
            "xT": _pack_tokens(xf[tok_lists[e]], CAP),
            "xsT": _pack_tokens(xf[e * TS:(e + 1) * TS], TS),
            "upw": _pack_weight(np.asarray(up_w[e], np.float32)),
            "dnw": _pack_weight(np.asarray(down_w[e], np.float32)),
            "supw": supw,
            "sdnw": sdnw,
            "upb": _pack_bias(up_b[e]),
            "supb": supb,
            "dnb": _pack_bias(down_b[e], scale=0.1),
            "sdnb": sdnb,
        })

    nc = _get_compiled()
    res = run_bass_kernel_spmd(nc, in_maps, list(range(E)))

    # ---- host combine: scatter-add expert outputs, add shared, normalize
    out = np.zeros((T, H), np.float32)
    for e in range(E):
        soT = np.asarray(res.results[e]["soT"], np.float32)   # [HC, 128, TS]
        out[e * TS:(e + 1) * TS] = soT.reshape(H, TS).T
    for e in range(E):
        tl = tok_lists[e]
        if len(tl):
            eoT = np.asarray(res.results[e]["eoT"], np.float32)  # [HC,128,CAP]
            eo = eoT.reshape(H, CAP)[:, :len(tl)].T              # [n, H]
            we = np.where(top_idx[tl, 0] == e,
                          wts[tl, 0], wts[tl, 1]).astype(np.float32)
            out[tl] += we[:, None] * eo
        ol = over_lists[e]
        if len(ol):  # overflow pairs: exact fp32 FFN on host
            u = xf[ol] @ np.asarray(up_w[e], np.float32) \
                + np.asarray(up_b[e], np.float32)
            eo = (_gelu(u) @ np.asarray(down_w[e], np.float32)
                  + np.asarray(down_b[e], np.float32)) * np.float32(0.1)
            we = np.where(top_idx[ol, 0] == e,
                          wts[ol, 0], wts[ol, 1]).astype(np.float32)
            out[ol] += we[:, None] * eo

    out /= (np.abs(out).max(-1, keepdims=True) + np.float32(1e-6))
    return out.reshape(B, S, H)
